# revision 1
# baseline (speedup 1.0000x reference)
"""ANI-style AEVComputer on 8 TRN2 NeuronCores (Bass/Tile).

Strategy
--------
Data-parallel over conformations: each of the 8 cores processes 2 of the 16
conformations end to end; no cross-core communication.

Per core, on device:
  *  d^2 matrix per conformation via one 9-wide TensorE matmul
     (A9=[c^2;1;c], B9=[1;c^2;-2c], d^2 = A9.T@B9); both conformations'
     geometry tiles are stacked on the partition axis (atoms 0-63 = conf 0,
     64-127 = conf 1) so every elementwise stage-R op covers both at once
  *  d = exp(ln(d2)/2), cutoff fns via ACT Sin (argument folded into [-pi,pi])
  *  radial AEV: 16 shifted gaussians * fc, scattered over species by one
     matmul into 8 joint bins (4 species x 2 conformations)
  *  angular AEV over atom pairs (j,k): host packs the live pairs of BOTH
     conformations into one stream, chunked 126 pairs at a time; all pair
     geometry is derived from the joint d/d^2/fc tiles by selection matmuls;
     ((1+cos(theta-shfz))/2)**32 uses cos(theta-shfz) = c*cz + s*sz and
     pow via exp(32*ln(h)); scatter over 20 joint bins (10 species-pairs x
     2 conformations) by matmul with per-pair one-hot weights, PSUM-accumulated
     across chunks.

The host precomputes integer-derived selection/one-hot tables only; all
floating-point geometry math runs on device.  Pairs (j,k) that cannot
contribute (no atom within the angular cutoff of both) are compacted out on
host; the kernel is compiled for the padded joint live-pair count K (cached).
"""
import sys

if '/opt/trn_rl_repo' not in sys.path:
    sys.path.insert(0, '/opt/trn_rl_repo')

import numpy as np
import ml_dtypes

import concourse.bass as bass
import concourse.tile as tile
from concourse import mybir
from concourse.bass_utils import run_bass_kernel_spmd

DT = mybir.dt
AF = mybir.ActivationFunctionType
ALU = mybir.AluOpType

# ---------------- walrus compat: one sync wait per instruction ----------------


def _split_multiwaits(nc):
    n = 0
    for f in nc.m.functions:
        for bb in f.blocks:
            insts = bb.instructions
            out = []
            changed = False
            for inst in insts:
                si = inst.sync_info
                waits = list(si.on_wait) if si is not None else []
                if len(waits) > 1:
                    changed = True
                    for w in waits[:-1]:
                        n += 1
                        out.append(mybir.InstNoOp(
                            name=f"mwsplit-{n}", engine=inst.engine, ins=[], outs=[],
                            sync_info=mybir.SyncInfo(on_wait=[w], on_update=[]),
                        ))
                    inst.sync_info = mybir.SyncInfo(
                        on_wait=[waits[-1]], on_update=list(si.on_update))
                out.append(inst)
            if changed:
                insts.clear()
                insts.extend(out)
    return n




def _install_drain_patch():
    from concourse.tile import TileContext
    from concourse.vector_clock import ScopedClock

    def _patched(self, tick_clock, wait_clock):
        nc = self.nc
        drain_inst = nc.sync.drain()
        wait_clock.add_sem_waits(
            drain_inst.ins, ScopedClock({None: tick_clock.global_clock}))
        si = drain_inst.ins.sync_info
        waits = list(si.on_wait) if si else []
        if len(waits) > 1:
            # leave one wait on the drain; spread the rest across engines so
            # they are satisfied in parallel before the all-engine barrier
            drain_inst.ins.sync_info = mybir.SyncInfo(
                on_wait=waits[:1], on_update=[])
            engs = [nc.vector, nc.scalar, nc.gpsimd, nc.tensor, nc.sync]
            for idx, wt in enumerate(waits[1:]):
                e = engs[idx % len(engs)]
                nop = e.nop(nofuse=True)
                nop.ins.sync_info = mybir.SyncInfo(on_wait=[wt], on_update=[])
        nc.all_engine_barrier()
        assert self.sems is not None
        popped = nc._tile_sem_poison_stack.pop()
        assert popped is self._sem_poison
        nc.clear_and_free_semaphores(list(self.sems.allocated().values()))
        nc.all_engine_barrier()

    TileContext._drain_and_barrier = _patched


_install_drain_patch()

# ---------------- problem constants ----------------
RCR, RCA = 5.2, 3.5
ETA_R, ETA_A, ZETA = 16.0, 8.0, 32.0
SHF_R = (0.9 + 0.26875 * np.arange(16)).astype(np.float64)
SHF_A = np.array([0.9, 1.55, 2.2, 2.85], np.float64)
SHF_Z = (np.pi / 16 + (np.pi / 8) * np.arange(8)).astype(np.float64)
NSP = 4
C, A = 16, 64
A2 = 2 * A                    # joint partition count (2 conformations)
NCORES, CPC = 8, 2
IDX_I, IDX_J = np.triu_indices(A, k=1)
P_FULL = IDX_I.size           # 2016
PCH = 126                     # pairs per chunk

_tbl = np.zeros((NSP, NSP), np.int64)
_k = 0
for _a in range(NSP):
    for _b in range(_a, NSP):
        _tbl[_a, _b] = _tbl[_b, _a] = _k
        _k += 1
NPAIR_T = _k                  # 10
NBIN = 2 * NPAIR_T            # 20 joint bins

DIAG = RCR + 1.0

_NC_CACHE = {}


def _build(K):
    """Per-core Bass graph; K = padded joint live-pair count."""
    sizes = []
    r = K
    while r > 0:
        sizes.append(min(PCH, r))
        r -= min(PCH, r)
    CH = len(sizes)
    nc = bass.Bass("TRN2", target_bir_lowering=False, debug=False)

    coords = nc.declare_dram_parameter("coords", [CPC, A, 3], DT.float32, isOutput=False)
    selit = nc.declare_dram_parameter("selit", [A2, K], DT.float32, isOutput=False)
    seljt = nc.declare_dram_parameter("seljt", [A2, K], DT.float32, isOutput=False)
    seljpf = nc.declare_dram_parameter("seljpf", [K, A], DT.float32, isOutput=False)
    ohp = nc.declare_dram_parameter("ohp", [K, NBIN], DT.bfloat16, isOutput=False)
    ohs = nc.declare_dram_parameter("ohs", [A2, 2 * NSP], DT.bfloat16, isOutput=False)
    out = nc.declare_dram_parameter("out", [CPC, A, 384], DT.float32, isOutput=True)

    CZ = np.cos(SHF_Z)
    SZ = np.sin(SHF_Z)

    with tile.TileContext(nc) as tc:
        with tc.tile_pool(name="cpool", bufs=1) as cpool, \
             tc.tile_pool(name="sbC", bufs=1) as sbC, \
             tc.tile_pool(name="sbK", bufs=5) as sbK, \
             tc.tile_pool(name="psG", bufs=2, space="PSUM") as psG, \
             tc.tile_pool(name="psA", bufs=1, space="PSUM") as psA:

            consts = {}

            def cst(val):
                v = float(val)
                if v not in consts:
                    t = cpool.tile([128, 1], DT.float32, tag=f"cst{len(consts)}",
                                   name=f"cst{len(consts)}")
                    nc.vector.memset(t[:], v)
                    consts[v] = t
                return consts[v]

            # joint eye * DIAG: partition p hits free column (p mod 64)
            iP = cpool.tile([A2, A], DT.float32)
            nc.gpsimd.iota(iP[:], [[0, A]], channel_multiplier=1,
                           allow_small_or_imprecise_dtypes=True)
            iF = cpool.tile([A2, A], DT.float32)
            nc.gpsimd.iota(iF[:], [[1, A]], channel_multiplier=0,
                           allow_small_or_imprecise_dtypes=True)
            iF2 = cpool.tile([A2, A], DT.float32)
            nc.vector.tensor_scalar(iF2[:], iF[:], A * 1.0, None, ALU.add)
            eye = cpool.tile([A2, A], DT.float32)
            t_eq = cpool.tile([A2, A], DT.float32)
            nc.vector.tensor_tensor(eye[:], iP[:], iF[:], ALU.is_equal)
            nc.vector.tensor_tensor(t_eq[:], iP[:], iF2[:], ALU.is_equal)
            nc.vector.tensor_tensor(eye[:], eye[:], t_eq[:], ALU.add)
            nc.vector.tensor_scalar(eye[:], eye[:], DIAG, None, ALU.mult)

            # shift-constant tiles expanded along free dims
            czq = cpool.tile([PCH, 8, A], DT.float32)
            szq = cpool.tile([PCH, 8, A], DT.float32)
            for z in range(8):
                nc.gpsimd.memset(czq[:, z, :], 0.475 * CZ[z])
                nc.gpsimd.memset(szq[:, z, :], 0.5 * SZ[z])
            shfa2q = cpool.tile([PCH, 4, A], DT.float32)
            for a in range(4):
                nc.gpsimd.memset(shfa2q[:, a, :], 2.0 * SHF_A[a])
            shfrq = cpool.tile([A2, 16, A], DT.float32)
            for rr in range(16):
                nc.gpsimd.memset(shfrq[:, rr, :], SHF_R[rr])

            # big selection tables (prefetched immediately)
            selI = sbC.tile([A2, K], DT.float32)
            nc.sync.dma_start(selI[:], selit[:])
            selJ = sbC.tile([A2, K], DT.float32)
            nc.scalar.dma_start(selJ[:], seljt[:])
            ohs_sb = sbC.tile([A2, 2 * NSP], DT.bfloat16)
            nc.gpsimd.dma_start(ohs_sb[:], ohs[:])

            # ---------------- stage R: joint geometry ----------------
            geo = sbC.tile([A2, 3, A], DT.float32)   # [d | dsqc | fcA] joint
            d_t, dsqc, fcA = geo[:, 0, :], geo[:, 1, :], geo[:, 2, :]
            for cc in range(CPC):
                A9 = sbC.tile([9, A], DT.float32, tag="A9", name=f"A9{cc}")
                B9 = sbC.tile([9, A], DT.float32, tag="B9", name=f"B9{cc}")
                ct = sbC.tile([3, A], DT.float32, tag="ct", name=f"ct{cc}")
                nc.sync.dma_start(ct[:], coords[cc].rearrange("a k -> k a"))
                nc.scalar.square(A9[0:3, :], ct[:])
                nc.vector.memset(B9[0:3, :], 1.0)
                m2ct = sbC.tile([3, A], DT.float32, tag="m2ct", name=f"m2ct{cc}")
                nc.vector.tensor_scalar(m2ct[:], ct[:], -2.0, None, ALU.mult)
                nc.sync.dma_start(A9[3:6, :], B9[0:3, :])
                nc.sync.dma_start(A9[6:9, :], coords[cc].rearrange("a k -> k a"))
                nc.sync.dma_start(B9[3:6, :], A9[0:3, :])
                nc.sync.dma_start(B9[6:9, :], m2ct[:])
                dsqp = psA.tile([A, A], DT.float32, tag=f"aev{2 * cc}", name=f"dsq{cc}")
                nc.tensor.matmul(dsqp[:], A9[:], B9[:], start=True, stop=True)
                nc.vector.tensor_scalar(dsqc[A * cc:A * (cc + 1), :], dsqp[:],
                                        0.0, None, ALU.max)

            lnd = sbC.tile([A2, A], DT.float32)
            nc.scalar.activation(lnd[:], dsqc, AF.Ln)
            nc.scalar.activation(d_t, lnd[:], AF.Exp, scale=0.5)
            nc.vector.tensor_tensor(d_t, d_t, eye[:], ALU.add)

            # cutoffs (joint): fc = mask * (0.5 + 0.5*sin(pi/2 - pi*d/rc))
            fcRq = sbC.tile([A2, A], DT.float32)
            for (dst, rc, s1, s2) in ((fcA, RCA, 0.5, 0.5), (fcRq[:], RCR, 0.125, 0.125)):
                dcl = sbC.tile([A2, A], DT.float32, tag="dcl", name=f"dcl{rc}")
                nc.vector.tensor_scalar(dcl[:], d_t, rc * 1.01, None, ALU.min)
                sn = sbC.tile([A2, A], DT.float32, tag="sn", name=f"sn{rc}")
                nc.scalar.activation(sn[:], dcl[:], AF.Sin,
                                     bias=cst(np.pi / 2)[:A2, 0:1], scale=-np.pi / rc)
                msk = sbC.tile([A2, A], DT.float32, tag="msk", name=f"msk{rc}")
                nc.vector.tensor_scalar(msk[:], d_t, rc, None, ALU.is_le)
                nc.vector.tensor_scalar(sn[:], sn[:], s1, s2, ALU.mult, ALU.add)
                nc.vector.tensor_tensor(dst, sn[:], msk[:], ALU.mult)

            # ---------------- radial (joint) ----------------
            rt = sbC.tile([A2, 16, A], DT.float32)
            nc.vector.tensor_tensor(
                rt[:], geo[:, 0:1, :].broadcast_to([A2, 16, A]), shfrq[:],
                ALU.subtract)
            nc.scalar.activation(rt[:], rt[:], AF.Square)
            nc.scalar.activation(rt[:], rt[:], AF.Exp, scale=-ETA_R)
            rtm = sbC.tile([A2, 16, A], DT.bfloat16)
            nc.gpsimd.tensor_tensor(
                rtm[:], rt[:],
                fcRq[:].rearrange("p (r i) -> p r i", r=1).broadcast_to([A2, 16, A]),
                ALU.mult)
            radsb = sbC.tile([2 * NSP, A, 16], DT.float32)
            for half in range(2):
                radp = psA.tile([2 * NSP, 8, A], DT.float32, tag="aev1",
                                name=f"radp{half}")
                nc.tensor.matmul(radp[:], ohs_sb[:], rtm[:, 8 * half:8 * (half + 1), :],
                                 start=True, stop=True)
                dst = radsb[:, :, 8 * half:8 * (half + 1)]
                src = radp[:].rearrange("s z i -> s i z")
                if half == 0:
                    nc.scalar.copy(dst, src)
                else:
                    nc.vector.tensor_copy(dst, src)
            for cc in range(CPC):
                nc.sync.dma_start(
                    out[cc, :, 0:A].rearrange("i (s r) -> s i r", s=NSP),
                    radsb[NSP * cc:NSP * (cc + 1)])

            # ---------------- angular over the joint pair stream ----------------
            aev = []
            for a in range(4):
                t = psA.tile([NBIN, 8, A], DT.float32, tag=f"aev{a}", name=f"aev{a}")
                aev.append(t)

            pos = 0
            for ch in range(CH):
                w = sizes[ch]
                sl = slice(pos, pos + w)
                pos += w
                gp1 = psG.tile([PCH, 3, A], DT.float32, tag="gp1", name=f"gp1_{ch}")
                nc.tensor.matmul(gp1[:w], selI[:, sl], geo[:], start=True, stop=True)
                gp2 = psG.tile([PCH, 3, A], DT.float32, tag="gp2", name=f"gp2_{ch}")
                nc.tensor.matmul(gp2[:w], selJ[:, sl], geo[:], start=True, stop=True)
                sb1 = sbK.tile([PCH, 3, A], DT.float32, tag="sb1", name=f"sb1_{ch}")
                nc.scalar.copy(sb1[:w], gp1[:w])
                sb2 = sbK.tile([PCH, 3, A], DT.float32, tag="sb2", name=f"sb2_{ch}")
                nc.vector.tensor_copy(sb2[:w], gp2[:w])
                d1, dq1, fc1 = sb1[:w, 0, :], sb1[:w, 1, :], sb1[:w, 2, :]
                d2, dq2, fc2 = sb2[:w, 0, :], sb2[:w, 1, :], sb2[:w, 2, :]

                jpf = sbK.tile([PCH, A], DT.float32, tag="jpf", name=f"jpf{ch}")
                nc.gpsimd.dma_start(jpf[:w], seljpf[sl])
                jm = sbK.tile([PCH, A], DT.float32, tag="jm", name=f"jm{ch}")
                nc.vector.tensor_tensor(jm[:w], dq1, jpf[:w], ALU.mult)
                djk2 = sbK.tile([PCH, 1], DT.float32, tag="djk2", name=f"djk2{ch}")
                nc.vector.tensor_reduce(djk2[:w], jm[:w], mybir.AxisListType.X, ALU.add)

                tsum = sbK.tile([PCH, A], DT.float32, tag="tsum", name=f"ts{ch}")
                nc.gpsimd.tensor_tensor(tsum[:w], d1, d2, ALU.add)
                nsum = sbK.tile([PCH, A], DT.float32, tag="nsum", name=f"ns{ch}")
                nc.gpsimd.tensor_tensor(nsum[:w], dq1, dq2, ALU.add)
                dn = sbK.tile([PCH, 2, A], DT.float32, tag="dn", name=f"dn{ch}")
                nc.gpsimd.tensor_tensor(dn[:w, 0, :], d1, d2, ALU.mult)

                nn = sbK.tile([PCH, A], DT.float32, tag="nn", name=f"nn{ch}")
                nc.vector.tensor_scalar(nn[:w], nsum[:w], djk2[:w, 0:1], 0.5,
                                        ALU.subtract, ALU.mult)
                lden = sbK.tile([PCH, A], DT.float32, tag="ld", name=f"ld{ch}")
                nc.scalar.activation(lden[:w], dn[:w, 0, :], AF.Ln)
                rcp = sbK.tile([PCH, A], DT.float32, tag="rcp", name=f"rcp{ch}")
                nc.scalar.activation(rcp[:w], lden[:w], AF.Exp, scale=-1.0)
                u = sbK.tile([PCH, A], DT.float32, tag="u", name=f"u{ch}")
                nc.vector.tensor_tensor(u[:w], nn[:w], rcp[:w], ALU.mult)
                nc.vector.tensor_scalar(u[:w], u[:w], 1.0, -1.0, ALU.min, ALU.max)
                usq = sbK.tile([PCH, A], DT.float32, tag="usq", name=f"usq{ch}")
                nc.vector.tensor_tensor(usq[:w], u[:w], u[:w], ALU.mult)
                nc.vector.tensor_scalar(dn[:w, 1, :], usq[:w], -0.9025, 1.0,
                                        ALU.mult, ALU.add)
                lss = sbK.tile([PCH, A], DT.float32, tag="lss", name=f"lss{ch}")
                nc.scalar.activation(lss[:w], dn[:w, 1, :], AF.Ln)
                ss = sbK.tile([PCH, A], DT.float32, tag="ss", name=f"ss{ch}")
                nc.scalar.activation(ss[:w], lss[:w], AF.Exp, scale=0.5)

                g = sbK.tile([PCH, A], DT.float32, tag="g", name=f"g{ch}")
                nc.vector.tensor_tensor(g[:w], fc1, fc2, ALU.mult)

                f2 = sbK.tile([PCH, 4, A], DT.float32, tag="f2", name=f"f2{ch}")
                nc.gpsimd.tensor_tensor(
                    f2[:w], tsum[:w].rearrange("p (a i) -> p a i", a=1
                                               ).broadcast_to([w, 4, A]),
                    shfa2q[:w], ALU.subtract)
                nc.scalar.activation(f2[:w], f2[:w], AF.Square)
                nc.scalar.activation(f2[:w], f2[:w], AF.Exp, scale=-2.0)
                f2g = sbK.tile([PCH, 4, A], DT.bfloat16, tag="f2g", name=f"f2g{ch}")
                nc.vector.tensor_tensor(
                    f2g[:w], f2[:w],
                    g[:w].rearrange("p (a i) -> p a i", a=1).broadcast_to([w, 4, A]),
                    ALU.mult)

                h = sbK.tile([PCH, 8, A], DT.float32, tag="h", name=f"h{ch}")
                th2 = sbK.tile([PCH, 8, A], DT.float32, tag="th2", name=f"th2{ch}")
                nc.vector.tensor_tensor(
                    h[:w], u[:w].rearrange("p (z i) -> p z i", z=1
                                           ).broadcast_to([w, 8, A]),
                    czq[:w], ALU.mult)
                nc.vector.tensor_tensor(
                    th2[:w], ss[:w].rearrange("p (z i) -> p z i", z=1
                                              ).broadcast_to([w, 8, A]),
                    szq[:w], ALU.mult)
                nc.vector.tensor_tensor(h[:w], h[:w], th2[:w], ALU.add)
                nc.vector.tensor_scalar(h[:w], h[:w], 0.5, 0.0, ALU.add, ALU.max)
                lnh = sbK.tile([PCH, 8, A], DT.float32, tag="lnh", name=f"lnh{ch}")
                nc.scalar.activation(lnh[:w], h[:w], AF.Ln)
                f1 = sbK.tile([PCH, 8, A], DT.bfloat16, tag="f1", name=f"f1{ch}")
                nc.scalar.activation(f1[:w], lnh[:w], AF.Exp, scale=ZETA)

                at = sbK.tile([PCH, 4, 8, A], DT.bfloat16, tag="at", name=f"at{ch}")
                nc.vector.tensor_tensor(
                    at[:w],
                    f1[:w].rearrange("p (a z) i -> p a z i", a=1
                                     ).broadcast_to([w, 4, 8, A]),
                    f2g[:w].rearrange("p a (z i) -> p a z i", z=1
                                      ).broadcast_to([w, 4, 8, A]),
                    ALU.mult)

                ohp_sb = sbK.tile([PCH, NBIN], DT.bfloat16, tag="ohp", name=f"ohp{ch}")
                nc.gpsimd.dma_start(ohp_sb[:w], ohp[sl])
                for a in range(4):
                    nc.tensor.matmul(aev[a][:], ohp_sb[:w], at[:w, a],
                                     start=(ch == 0), stop=(ch == CH - 1))

            aevsb = sbC.tile([NBIN, A, 32], DT.float32)
            for a in range(4):
                dst = aevsb[:, :, 8 * a:8 * (a + 1)]
                src = aev[a][:].rearrange("t z i -> t i z")
                nc.scalar.copy(dst, src)
            for cc in range(CPC):
                nc.scalar.dma_start(
                    out[cc, :, A:].rearrange("i (t z) -> t i z", t=NPAIR_T),
                    aevsb[NPAIR_T * cc:NPAIR_T * (cc + 1)])

    _split_multiwaits(nc)
    return nc


# ---------------- host side ----------------

def _prep(species, coordinates):
    sp = np.clip(np.asarray(species).astype(np.int64), 0, NSP - 1)
    co = np.ascontiguousarray(np.asarray(coordinates), dtype=np.float32)
    d2 = ((co[:, :, None, :].astype(np.float64) - co[:, None, :, :]) ** 2).sum(-1)
    D = np.sqrt(d2)
    for c in range(C):
        np.fill_diagonal(D[c], 1e9)
    near = D < (RCA + 0.02)
    live = (near[:, :, IDX_I] & near[:, :, IDX_J]).any(axis=1)   # (C, P)
    counts = live.sum(axis=1)
    K = int(max(counts[2 * k] + counts[2 * k + 1] for k in range(NCORES)))
    K = max(PCH, int(np.ceil(K / PCH)) * PCH)

    ar = np.arange(A)
    per_core = []
    for k in range(NCORES):
        streams = []
        for cc in range(CPC):
            c = 2 * k + cc
            idx = np.nonzero(live[c])[0]
            streams.append(np.stack([np.full_like(idx, cc), idx], axis=1))
        st = np.concatenate(streams, axis=0)
        nlive = st.shape[0]
        if nlive == 0:
            st = np.zeros((1, 2), np.int64)
        lmask = np.concatenate([np.ones(nlive), np.zeros(K - nlive)])
        npad = K - st.shape[0]
        if npad:
            st = np.concatenate([st, np.tile(st[:1], (npad, 1))], axis=0)
        ccs, pidx = st[:, 0], st[:, 1]
        Isel, Jsel = IDX_I[pidx], IDX_J[pidx]
        rowI = A * ccs + Isel                     # rows in the joint tables
        rowJ = A * ccs + Jsel
        selit = (np.arange(A2)[:, None] == rowI[None, :]).astype(np.float32)
        seljt = (np.arange(A2)[:, None] == rowJ[None, :]).astype(np.float32)
        seljpf = (ar[None, :] == Jsel[:, None]).astype(np.float32)   # (K, A)
        carr = 2 * k + ccs
        pid = NPAIR_T * ccs + _tbl[sp[carr, Isel], sp[carr, Jsel]]
        ohpv = (2.0 * (pid[:, None] == np.arange(NBIN)) * lmask[:, None]
                ).astype(ml_dtypes.bfloat16)                          # (K, 20)
        ohsv = np.zeros((A2, 2 * NSP), np.float32)
        for cc in range(CPC):
            ohsv[A * cc:A * (cc + 1), NSP * cc:NSP * (cc + 1)] = (
                sp[2 * k + cc][:, None] == np.arange(NSP))
        per_core.append({
            "coords": np.ascontiguousarray(co[2 * k:2 * k + 2]),
            "selit": selit, "seljt": seljt, "seljpf": seljpf,
            "ohp": ohpv, "ohs": ohsv.astype(ml_dtypes.bfloat16),
        })
    return K, per_core


def _run(species, coordinates, trace=False):
    K, in_maps = _prep(species, coordinates)
    if K not in _NC_CACHE:
        _NC_CACHE[K] = _build(K)
    nc = _NC_CACHE[K]
    res = run_bass_kernel_spmd(nc, in_maps, core_ids=list(range(NCORES)), trace=trace)
    outs = np.concatenate([res.results[k]["out"] for k in range(NCORES)], axis=0)
    return outs.astype(np.float32), res


def kernel(species, coordinates):
    out, _ = _run(species, coordinates, trace=False)
    return out



# revision 12
# speedup vs baseline: 2.0227x; 2.0227x over previous
"""ANI-style AEVComputer on 8 TRN2 NeuronCores (Bass/Tile).

Strategy
--------
Data-parallel over conformations: each of the 8 cores processes 2 of the 16
conformations (host pairs large-triple-count confs with small ones for
balance); no cross-core communication.

Angular part: instead of iterating (pair x all 64 atoms) like the previous
version (~30x wasted lanes), the host enumerates the exact (center i, j<k)
triples with both neighbors inside the angular cutoff (~1.5k per core),
ships gathered coordinates [Ri|Rj|Rk] per triple (pure indexing of the
input, no host float math on the values), and the device computes the
32 angular basis values per triple with triples laid out as
(partition, chunk) so every elementwise op covers ALL chunks in one
instruction.  Scatter to (conf-atom, species-pair) bins is one 128-wide
one-hot matmul per 128-triple chunk, PSUM-accumulated; the one-hots are
built on device from shipped integer ids via iota + is_equal.

Radial part: joint (2 conf x 64 atom) d-matrix via the 9-wide TensorE
matmul trick, 16 shifted gaussians * cutoff, one-hot species scatter by
matmul - same as before but with Sqrt instead of Ln/Exp and no diagonal
fixup (the d=0 self term contributes < 1e-6).
"""
import sys

if '/opt/trn_rl_repo' not in sys.path:
    sys.path.insert(0, '/opt/trn_rl_repo')

import numpy as np
import ml_dtypes

import concourse.bass as bass
import concourse.tile as tile
from concourse import mybir
from concourse.bass_utils import run_bass_kernel_spmd

DT = mybir.dt
AF = mybir.ActivationFunctionType
ALU = mybir.AluOpType

# ---------------- walrus compat: one sync wait per instruction ----------------


def _split_multiwaits(nc):
    n = 0
    for f in nc.m.functions:
        for bb in f.blocks:
            insts = bb.instructions
            out = []
            changed = False
            for inst in insts:
                si = inst.sync_info
                waits = list(si.on_wait) if si is not None else []
                if len(waits) > 1:
                    changed = True
                    for w in waits[:-1]:
                        n += 1
                        out.append(mybir.InstNoOp(
                            name=f"mwsplit-{n}", engine=inst.engine, ins=[], outs=[],
                            sync_info=mybir.SyncInfo(on_wait=[w], on_update=[]),
                        ))
                    inst.sync_info = mybir.SyncInfo(
                        on_wait=[waits[-1]], on_update=list(si.on_update))
                out.append(inst)
            if changed:
                insts.clear()
                insts.extend(out)
    return n


def _install_drain_patch():
    from concourse.tile import TileContext
    from concourse.vector_clock import ScopedClock

    def _patched(self, tick_clock, wait_clock):
        nc = self.nc
        drain_inst = nc.sync.drain()
        wait_clock.add_sem_waits(
            drain_inst.ins, ScopedClock({None: tick_clock.global_clock}))
        si = drain_inst.ins.sync_info
        waits = list(si.on_wait) if si else []
        if len(waits) > 1:
            drain_inst.ins.sync_info = mybir.SyncInfo(
                on_wait=waits[:1], on_update=[])
            engs = [nc.vector, nc.scalar, nc.gpsimd, nc.tensor, nc.sync]
            for idx, wt in enumerate(waits[1:]):
                e = engs[idx % len(engs)]
                nop = e.nop(nofuse=True)
                nop.ins.sync_info = mybir.SyncInfo(on_wait=[wt], on_update=[])
        nc.all_engine_barrier()
        assert self.sems is not None
        popped = nc._tile_sem_poison_stack.pop()
        assert popped is self._sem_poison
        nc.clear_and_free_semaphores(list(self.sems.allocated().values()))
        nc.all_engine_barrier()

    TileContext._drain_and_barrier = _patched


_install_drain_patch()

# ---------------- problem constants ----------------
RCR, RCA = 5.2, 3.5
SHF_R = (0.9 + 0.26875 * np.arange(16)).astype(np.float64)
SHF_A = np.array([0.9, 1.55, 2.2, 2.85], np.float64)
SHF_Z = (np.pi / 16 + (np.pi / 8) * np.arange(8)).astype(np.float64)
NSP = 4
C, A = 16, 64
A2 = 2 * A
NCORES, CPC = 8, 2

_tbl = np.zeros((NSP, NSP), np.int64)
_k = 0
for _a in range(NSP):
    for _b in range(_a, NSP):
        _tbl[_a, _b] = _tbl[_b, _a] = _k
        _k += 1
NPAIR_T = _k                  # 10

# lane-constant table columns
_LN_SHFA2 = 0                 # 4: 2*SHF_A
_LN_CZ = 4                    # 8: 0.475*cos(SHF_Z)
_LN_SZ = 12                   # 8: 0.5*sin(SHF_Z)
_LN_SHFR = 20                 # 16: SHF_R
_LN_PI2 = 36                  # 1: pi/2
_LN_LN2 = 37                  # 1: ln(2)
_LN_N = 38

_NC_CACHE = {}


def _build(NCH):
    """Per-core Bass graph; NCH = number of 128-triple chunks."""
    nc = bass.Bass("TRN2", target_bir_lowering=False, debug=False)

    coords = nc.declare_dram_parameter("coords", [CPC, A, 3], DT.float32, isOutput=False)
    rjk = nc.declare_dram_parameter("rjk", [A2, NCH, 9], DT.float32, isOutput=False)
    meta = nc.declare_dram_parameter("meta", [A2, 2, NCH], DT.float32, isOutput=False)
    lanes = nc.declare_dram_parameter("lanes", [A2, _LN_N], DT.float32, isOutput=False)
    ohs = nc.declare_dram_parameter("ohs", [A2, 2 * NSP], DT.bfloat16, isOutput=False)
    out = nc.declare_dram_parameter("out", [CPC, A, 384], DT.float32, isOutput=True)

    with tile.TileContext(nc) as tc:
        with tc.tile_pool(name="sb", bufs=1) as sb, \
             tc.tile_pool(name="ps", bufs=1, space="PSUM") as ps:

            # ---------- input DMAs + iota (issued first; transfers overlap) ----
            rjk_sb = sb.tile([A2, NCH, 9], DT.float32)
            h1 = (NCH + 1) // 2
            nc.sync.dma_start(rjk_sb[:, 0:h1, :], rjk[:, 0:h1, :])
            nc.scalar.dma_start(rjk_sb[:, h1:NCH, :], rjk[:, h1:NCH, :])
            meta_sb = sb.tile([A2, 2, NCH], DT.float32)
            nc.gpsimd.dma_start(meta_sb[:], meta[:])
            lanes_sb = sb.tile([A2, _LN_N], DT.float32)
            nc.sync.dma_start(lanes_sb[:], lanes[:])
            ohs_sb = sb.tile([A2, 2 * NSP], DT.bfloat16)
            nc.gpsimd.dma_start(ohs_sb[:], ohs[:])
            iotaF = sb.tile([A2, 128], DT.float32)
            nc.gpsimd.iota(iotaF[:], [[1, 128]], channel_multiplier=0,
                           allow_small_or_imprecise_dtypes=True)

            def lane(c0, n, w):
                return lanes_sb[:, c0:c0 + n].rearrange(
                    "p (c k) -> p c k", c=1).broadcast_to([A2, w, n])

            # ---------- radial phase (joint 2 confs) ----------
            dsqc = sb.tile([A2, A], DT.float32)
            for cc in range(CPC):
                A9 = sb.tile([9, A], DT.float32, tag="A9", name=f"A9{cc}")
                B9 = sb.tile([9, A], DT.float32, tag="B9", name=f"B9{cc}")
                ct = sb.tile([3, A], DT.float32, tag="ct", name=f"ct{cc}")
                m2ct = sb.tile([3, A], DT.float32, tag="m2ct", name=f"m2ct{cc}")
                nc.sync.dma_start(ct[:], coords[cc].rearrange("a k -> k a"))
                nc.scalar.square(A9[0:3, :], ct[:])
                nc.gpsimd.memset(B9[0:3, :], 1.0)
                nc.vector.tensor_scalar(m2ct[:], ct[:], -2.0, None, ALU.mult)
                nc.sync.dma_start(A9[3:6, :], B9[0:3, :])
                nc.sync.dma_start(A9[6:9, :], coords[cc].rearrange("a k -> k a"))
                nc.sync.dma_start(B9[3:6, :], A9[0:3, :])
                nc.sync.dma_start(B9[6:9, :], m2ct[:])
                dsqp = ps.tile([A, A], DT.float32, tag=f"dsq{cc}", name=f"dsq{cc}")
                nc.tensor.matmul(dsqp[:], A9[:], B9[:], start=True, stop=True)
                nc.vector.tensor_scalar(dsqc[A * cc:A * (cc + 1), :], dsqp[:],
                                        0.0, None, ALU.max)

            d_t = sb.tile([A2, A], DT.float32)
            nc.scalar.activation(d_t[:], dsqc[:], AF.Sqrt)

            dclR = sb.tile([A2, A], DT.float32)
            nc.vector.tensor_scalar(dclR[:], d_t[:], RCR * 1.01, None, ALU.min)
            snR = sb.tile([A2, A], DT.float32)
            nc.scalar.activation(snR[:], dclR[:], AF.Sin,
                                 bias=lanes_sb[:, _LN_PI2:_LN_PI2 + 1],
                                 scale=-np.pi / RCR)
            mskR = sb.tile([A2, A], DT.float32)
            nc.gpsimd.tensor_scalar(mskR[:], d_t[:], RCR, None, ALU.is_le)
            fchR = sb.tile([A2, A], DT.float32)
            nc.vector.tensor_scalar(fchR[:], snR[:], 0.125, 0.125, ALU.mult, ALU.add)
            fcR = sb.tile([A2, A], DT.float32)
            nc.gpsimd.tensor_tensor(fcR[:], fchR[:], mskR[:], ALU.mult)

            rsub = sb.tile([A2, 16, A], DT.float32)
            nc.vector.tensor_tensor(
                rsub[:],
                d_t[:].rearrange("p (r i) -> p r i", r=1).broadcast_to([A2, 16, A]),
                lanes_sb[:, _LN_SHFR:_LN_SHFR + 16].rearrange(
                    "p (r i) -> p r i", i=1).broadcast_to([A2, 16, A]),
                ALU.subtract)
            nc.vector.tensor_tensor(rsub[:], rsub[:], rsub[:], ALU.mult)
            rte = sb.tile([A2, 16, A], DT.float32)
            nc.scalar.activation(rte[:], rsub[:], AF.Exp, scale=-16.0)
            rtf = sb.tile([A2, 16, A], DT.bfloat16)
            nc.vector.tensor_tensor(
                rtf[:], rte[:],
                fcR[:].rearrange("p (r i) -> p r i", r=1).broadcast_to([A2, 16, A]),
                ALU.mult)

            radsb = sb.tile([2 * NSP, A, 16], DT.float32)
            for half in range(2):
                radp = ps.tile([2 * NSP, 8, A], DT.float32, tag="radp",
                               name=f"radp{half}")
                nc.tensor.matmul(radp[:], ohs_sb[:], rtf[:, 8 * half:8 * (half + 1), :],
                                 start=True, stop=True)
                dst = radsb[:, :, 8 * half:8 * (half + 1)]
                src = radp[:].rearrange("s z i -> s i z")
                if half == 0:
                    nc.scalar.copy(dst, src)
                else:
                    nc.vector.tensor_copy(dst, src)
            for cc in range(CPC):
                nc.sync.dma_start(
                    out[cc, :, 0:A].rearrange("i (s r) -> s i r", s=NSP),
                    radsb[NSP * cc:NSP * (cc + 1)])

            # ---------- angular phase: triple stream ----------
            # geometry: v1 = Ri - Rj, v2 = Ri - Rk
            v12 = sb.tile([A2, NCH, 2, 3], DT.float32)
            nc.vector.tensor_tensor(
                v12[:],
                rjk_sb[:, :, 0:3].rearrange("p c (u x) -> p c u x", u=1
                                            ).broadcast_to([A2, NCH, 2, 3]),
                rjk_sb[:, :, 3:9].rearrange("p c (u x) -> p c u x", u=2),
                ALU.subtract)
            sq6 = sb.tile([A2, NCH, 2, 3], DT.float32)
            nc.gpsimd.tensor_tensor(sq6[:], v12[:], v12[:], ALU.mult)
            dm3 = sb.tile([A2, NCH, 3], DT.float32)
            nc.vector.tensor_tensor(dm3[:], v12[:, :, 0, :], v12[:, :, 1, :],
                                    ALU.mult)
            dq2 = sb.tile([A2, NCH, 2], DT.float32)
            nc.vector.tensor_reduce(dq2[:], sq6[:], mybir.AxisListType.X, ALU.add)
            dot = sb.tile([A2, NCH, 1], DT.float32)
            nc.vector.tensor_reduce(dot[:], dm3[:], mybir.AxisListType.X, ALU.add)

            d2l = sb.tile([A2, NCH, 2], DT.float32)
            nc.scalar.activation(d2l[:], dq2[:], AF.Sqrt)
            prod = sb.tile([A2, NCH, 1], DT.float32)
            nc.vector.tensor_tensor(prod[:], d2l[:, :, 0:1], d2l[:, :, 1:2], ALU.mult)
            tsum = sb.tile([A2, NCH, 1], DT.float32)
            nc.gpsimd.tensor_tensor(tsum[:], d2l[:, :, 0:1], d2l[:, :, 1:2], ALU.add)

            rcp = sb.tile([A2, NCH, 1], DT.float32)
            nc.vector.reciprocal(rcp[:], prod[:])
            u = sb.tile([A2, NCH, 1], DT.float32)
            nc.vector.tensor_tensor(u[:], dot[:], rcp[:], ALU.mult)
            usq = sb.tile([A2, NCH, 1], DT.float32)
            nc.gpsimd.tensor_tensor(usq[:], u[:], u[:], ALU.mult)
            ssarg = sb.tile([A2, NCH, 1], DT.float32)
            nc.gpsimd.tensor_scalar(ssarg[:], usq[:], -0.9025, 1.0, ALU.mult, ALU.add)
            ss = sb.tile([A2, NCH, 1], DT.float32)
            nc.scalar.activation(ss[:], ssarg[:], AF.Sqrt)

            # angular cutoffs for both neighbors
            dcl2 = sb.tile([A2, NCH, 2], DT.float32)
            nc.vector.tensor_scalar(dcl2[:], d2l[:], RCA * 1.01, None, ALU.min)
            sn2 = sb.tile([A2, NCH, 2], DT.float32)
            nc.scalar.activation(sn2[:], dcl2[:], AF.Sin,
                                 bias=lanes_sb[:, _LN_PI2:_LN_PI2 + 1],
                                 scale=-np.pi / RCA)
            msk2 = sb.tile([A2, NCH, 2], DT.float32)
            nc.gpsimd.tensor_scalar(msk2[:], d2l[:], RCA, None, ALU.is_le)
            fch2 = sb.tile([A2, NCH, 2], DT.float32)
            nc.vector.tensor_scalar(fch2[:], sn2[:], 0.5, 0.5, ALU.mult, ALU.add)
            g2 = sb.tile([A2, NCH, 2], DT.float32)
            nc.gpsimd.tensor_tensor(g2[:], fch2[:], msk2[:], ALU.mult)
            g = sb.tile([A2, NCH, 1], DT.float32)
            nc.vector.tensor_tensor(g[:], g2[:, :, 0:1], g2[:, :, 1:2], ALU.mult)

            # f2 = 2*exp(-2*(tsum - 2*shfa)^2)   (x2 folded in via bias=ln2)
            am = sb.tile([A2, NCH, 4], DT.float32)
            nc.gpsimd.tensor_tensor(
                am[:], tsum[:].broadcast_to([A2, NCH, 4]),
                lane(_LN_SHFA2, 4, NCH), ALU.subtract)
            nc.vector.tensor_tensor(am[:], am[:], am[:], ALU.mult)
            f2 = sb.tile([A2, NCH, 4], DT.float32)
            nc.scalar.activation(f2[:], am[:], AF.Exp,
                                 bias=lanes_sb[:, _LN_LN2:_LN_LN2 + 1],
                                 scale=-2.0)
            f2g = sb.tile([A2, NCH, 4], DT.bfloat16)
            nc.vector.tensor_tensor(f2g[:], f2[:], g[:].broadcast_to([A2, NCH, 4]),
                                    ALU.mult)

            # f1 = ((1 + cos'(theta - shfz))/2)^32 via h = 0.5 + u*cz' + ss*sz'
            hc = sb.tile([A2, NCH, 8], DT.float32)
            nc.vector.tensor_tensor(hc[:], u[:].broadcast_to([A2, NCH, 8]),
                                    lane(_LN_CZ, 8, NCH), ALU.mult)
            hs = sb.tile([A2, NCH, 8], DT.float32)
            nc.gpsimd.tensor_tensor(hs[:], ss[:].broadcast_to([A2, NCH, 8]),
                                    lane(_LN_SZ, 8, NCH), ALU.mult)
            nc.vector.tensor_tensor(hc[:], hc[:], hs[:], ALU.add)
            nc.vector.tensor_scalar(hc[:], hc[:], 0.5, 0.0, ALU.add, ALU.max)
            lnh = sb.tile([A2, NCH, 8], DT.float32)
            nc.scalar.activation(lnh[:], hc[:], AF.Ln)
            f1 = sb.tile([A2, NCH, 8], DT.bfloat16)
            nc.scalar.activation(f1[:], lnh[:], AF.Exp, scale=32.0)

            at = sb.tile([A2, NCH, 32], DT.bfloat16)
            nc.vector.tensor_tensor(
                at[:].rearrange("p c (a z) -> p c a z", a=4),
                f1[:].rearrange("p c (a z) -> p c a z", a=1
                                ).broadcast_to([A2, NCH, 4, 8]),
                f2g[:].rearrange("p c (a z) -> p c a z", z=1
                                 ).broadcast_to([A2, NCH, 4, 8]),
                ALU.mult)

            # scatter one-hots from shipped ids
            ohci = sb.tile([A2, NCH, 128], DT.bfloat16)
            nc.vector.tensor_tensor(
                ohci[:],
                meta_sb[:, 0, :].rearrange("p (c m) -> p c m", m=1
                                           ).broadcast_to([A2, NCH, 128]),
                iotaF[:].rearrange("p (c m) -> p c m", c=1
                                   ).broadcast_to([A2, NCH, 128]),
                ALU.is_equal)
            ohsp = sb.tile([A2, NCH, NPAIR_T], DT.bfloat16)
            nc.vector.tensor_tensor(
                ohsp[:],
                meta_sb[:, 1, :].rearrange("p (c m) -> p c m", m=1
                                           ).broadcast_to([A2, NCH, NPAIR_T]),
                iotaF[:, 0:NPAIR_T].rearrange("p (c m) -> p c m", c=1
                                              ).broadcast_to([A2, NCH, NPAIR_T]),
                ALU.is_equal)
            at320 = sb.tile([A2, NCH, NPAIR_T, 32], DT.bfloat16)
            nc.vector.tensor_tensor(
                at320[:],
                at[:].rearrange("p c (s w) -> p c s w", s=1
                                ).broadcast_to([A2, NCH, NPAIR_T, 32]),
                ohsp[:].rearrange("p c (s w) -> p c s w", w=1
                                  ).broadcast_to([A2, NCH, NPAIR_T, 32]),
                ALU.mult)

            angp = ps.tile([A2, NPAIR_T * 32], DT.float32, tag="angp")
            for ch in range(NCH):
                nc.tensor.matmul(angp[:], ohci[:, ch, :], at320[:, ch],
                                 start=(ch == 0), stop=(ch == NCH - 1))

            angsb = sb.tile([A2, NPAIR_T * 32], DT.float32)
            nc.scalar.copy(angsb[:, 0:160], angp[:, 0:160])
            nc.vector.tensor_copy(angsb[:, 160:320], angp[:, 160:320])
            engs = [nc.sync, nc.gpsimd, nc.scalar, nc.sync]
            k = 0
            for cc in range(CPC):
                for rh in range(2):
                    r0 = 32 * rh
                    engs[k % 4].dma_start(
                        out[cc, r0:r0 + 32, A:384],
                        angsb[A * cc + r0:A * cc + r0 + 32, :])
                    k += 1

    _split_multiwaits(nc)
    return nc


# ---------------- host side ----------------

def _prep(species, coordinates):
    sp = np.clip(np.asarray(species).astype(np.int64), 0, NSP - 1)
    co = np.ascontiguousarray(np.asarray(coordinates), dtype=np.float32)
    d2 = ((co[:, :, None, :].astype(np.float64) - co[:, None, :, :]) ** 2).sum(-1)
    D = np.sqrt(d2)
    for c in range(C):
        np.fill_diagonal(D[c], 1e9)
    near = D < (RCA + 0.02)

    # enumerate (center, j<k) triples per conformation
    tri = []
    for c in range(C):
        Is, Js, Ks = [], [], []
        for i in range(A):
            nz = np.nonzero(near[c, i])[0]
            m = nz.size
            if m >= 2:
                jj, kk = np.triu_indices(m, k=1)
                Is.append(np.full(jj.size, i, np.int64))
                Js.append(nz[jj])
                Ks.append(nz[kk])
        if Is:
            tri.append((np.concatenate(Is), np.concatenate(Js), np.concatenate(Ks)))
        else:
            tri.append((np.zeros(0, np.int64),) * 3)

    counts = np.array([t[0].size for t in tri])
    order = np.argsort(-counts)
    confs = [(int(order[k]), int(order[15 - k])) for k in range(NCORES)]

    NCH = max(1, int(np.ceil(max(counts[ca] + counts[cb] for ca, cb in confs) / 128)))
    KT = NCH * 128

    lane_row = np.zeros(_LN_N, np.float64)
    lane_row[_LN_SHFA2:_LN_SHFA2 + 4] = 2.0 * SHF_A
    lane_row[_LN_CZ:_LN_CZ + 8] = 0.475 * np.cos(SHF_Z)
    lane_row[_LN_SZ:_LN_SZ + 8] = 0.5 * np.sin(SHF_Z)
    lane_row[_LN_SHFR:_LN_SHFR + 16] = SHF_R
    lane_row[_LN_PI2] = np.pi / 2
    lane_row[_LN_LN2] = np.log(2.0)
    lanes_t = np.tile(lane_row.astype(np.float32), (A2, 1))

    pad_rjk = np.array([0, 0, 0, 60, 0, 0, 0, 60, 0], np.float32)

    per_core = []
    for k in range(NCORES):
        ca, cb = confs[k]
        rjk_l, ci_l, spid_l = [], [], []
        for cc, c in enumerate((ca, cb)):
            I, J, K = tri[c]
            if I.size:
                rjk_l.append(np.concatenate(
                    [co[c, I], co[c, J], co[c, K]], axis=1))
                ci_l.append(A * cc + I)
                spid_l.append(_tbl[sp[c, J], sp[c, K]])
        T = sum(x.size for x in ci_l)
        rjk_f = np.full((KT, 9), 0, np.float32)
        rjk_f[:] = pad_rjk
        ci_f = np.zeros(KT, np.float32)
        spid_f = np.zeros(KT, np.float32)
        if T:
            rjk_f[:T] = np.concatenate(rjk_l, axis=0)
            ci_f[:T] = np.concatenate(ci_l).astype(np.float32)
            spid_f[:T] = np.concatenate(spid_l).astype(np.float32)
        # triple t = ch*128 + p  ->  tile [p, ch]
        rjk_t = rjk_f.reshape(NCH, 128, 9).transpose(1, 0, 2)
        meta_t = np.stack([ci_f.reshape(NCH, 128).T,
                           spid_f.reshape(NCH, 128).T], axis=1)  # (128, 2, NCH)

        ohsv = np.zeros((A2, 2 * NSP), np.float32)
        for cc, c in enumerate((ca, cb)):
            ohsv[A * cc:A * (cc + 1), NSP * cc:NSP * (cc + 1)] = (
                sp[c][:, None] == np.arange(NSP))
        per_core.append({
            "coords": np.ascontiguousarray(np.stack([co[ca], co[cb]])),
            "rjk": np.ascontiguousarray(rjk_t),
            "meta": np.ascontiguousarray(meta_t),
            "lanes": lanes_t,
            "ohs": ohsv.astype(ml_dtypes.bfloat16),
        })
    return NCH, per_core, confs


def _run(species, coordinates, trace=False):
    NCH, in_maps, confs = _prep(species, coordinates)
    if NCH not in _NC_CACHE:
        _NC_CACHE[NCH] = _build(NCH)
    nc = _NC_CACHE[NCH]
    res = run_bass_kernel_spmd(nc, in_maps, core_ids=list(range(NCORES)), trace=trace)
    full = np.empty((C, A, 384), np.float32)
    for k in range(NCORES):
        o = res.results[k]["out"]
        full[confs[k][0]] = o[0]
        full[confs[k][1]] = o[1]
    return full, res


def kernel(species, coordinates):
    out, _ = _run(species, coordinates, trace=False)
    return out
